# revision 2
# baseline (speedup 1.0000x reference)
"""Trainium2 Bass kernel for conv-attention (B=8, N=3136, C=192, 4 heads).

Sharding: data-parallel over batch, 1 batch element per NeuronCore (8 cores).
Per core: depthwise convs on DVE (scalar_tensor_tensor FMA taps), folded
pointwise+BN+projection matmuls on PE (bf16), softmax exp on ACT straight
from PSUM, PV with an augmented ones-column for the denominator, final
divide on DVE. Host does weight folding, padding/transpose, and unshard.
"""

import numpy as np
import ml_dtypes

import concourse.bass as bass
import concourse.bacc as bacc
import concourse.mybir as mybir
import concourse.tile as tile
from concourse.bass_utils import run_bass_kernel_spmd

BF16 = ml_dtypes.bfloat16

C = 192
H = W = 56
N = H * W            # 3136
MO = 28
M = MO * MO          # 784
NH = 4
HD = C // NH         # 48
EPS = 1e-5
PAD = 58             # padded image row stride
PADN = PAD * PAD     # 3364
XBASE = 64           # image offset inside the SBUF x buffer
XW = XBASE + PADN + XBASE  # 3492
CHUNK = 448          # 8 image rows per projection/attention n-chunk
NCH = 7              # n chunks
MT = 112             # m tile
NMT = 7              # m tiles
VW = NH * (HD + 1)   # 196, v with interleaved ones columns

_CACHE = {}


def _build_bass():
    fp32 = mybir.dt.float32
    bf16 = mybir.dt.bfloat16
    nc = bacc.Bacc(None)

    # ---- external I/O (per core) ----
    xe = nc.dram_tensor("xe", [128, XW], bf16, kind="ExternalInput")
    xo = nc.dram_tensor("xo", [128, XW], bf16, kind="ExternalInput")
    xe2 = nc.dram_tensor("xe2", [64, XW], bf16, kind="ExternalInput")
    xo2 = nc.dram_tensor("xo2", [64, XW], bf16, kind="ExternalInput")
    lqA = nc.dram_tensor("lqA", [128, 256], bf16, kind="ExternalInput")
    lqB = nc.dram_tensor("lqB", [65, 256], bf16, kind="ExternalInput")
    lkA = nc.dram_tensor("lkA", [128, 256], bf16, kind="ExternalInput")
    lkB = nc.dram_tensor("lkB", [65, 256], bf16, kind="ExternalInput")
    rvA = nc.dram_tensor("rvA", [128, VW], bf16, kind="ExternalInput")
    rvB = nc.dram_tensor("rvB", [65, VW], bf16, kind="ExternalInput")
    wq = nc.dram_tensor("wq", [192, 9], fp32, kind="ExternalInput")
    wkv = nc.dram_tensor("wkv", [192, 9], fp32, kind="ExternalInput")
    oa = nc.dram_tensor("oa", [96, N], fp32, kind="ExternalOutput")
    ob = nc.dram_tensor("ob", [96, N], fp32, kind="ExternalOutput")

    AF = mybir.ActivationFunctionType
    AL = mybir.AluOpType

    with tile.TileContext(nc) as tc:
        with (
            tc.tile_pool(name="xbuf", bufs=1) as xbuf,
            tc.tile_pool(name="wt", bufs=1) as wt,
            tc.tile_pool(name="z", bufs=1) as zp,
            tc.tile_pool(name="qk", bufs=1) as qk,
            tc.tile_pool(name="vs", bufs=1) as vsp,
            tc.tile_pool(name="pss", bufs=2, space="PSUM") as pss,
            tc.tile_pool(name="pso", bufs=4, space="PSUM") as pso,
            tc.tile_pool(name="pt", bufs=4) as ptp,
            tc.tile_pool(name="und", bufs=1) as undp,
            tc.tile_pool(name="tmpq", bufs=1) as tmpp,
            tc.tile_pool(name="dnp", bufs=2) as denp,
            tc.tile_pool(name="rcbp", bufs=4) as rcbpool,
            tc.tile_pool(name="fin", bufs=1) as finp,
            tc.tile_pool(name="drp", bufs=2, space="DRAM") as drp,
            tc.tile_pool(name="fin2", bufs=2) as fin2,
        ):
            # ---- load inputs ----
            wqt = wt.tile([128, 9], mybir.dt.float32, tag="wq")
            wqt2 = wt.tile([64, 9], mybir.dt.float32, tag="wq2")
            wkt = wt.tile([128, 9], mybir.dt.float32, tag="wk")
            wkt2 = wt.tile([64, 9], mybir.dt.float32, tag="wk2")
            nc.sync.dma_start(wkt[:], wkv[0:128, :])
            nc.sync.dma_start(wkt2[:], wkv[128:192, :])
            xeA = xbuf.tile([128, XW], bf16, tag="xeA")
            xoA = xbuf.tile([128, XW], bf16, tag="xoA")
            xeB = xbuf.tile([64, XW], bf16, tag="xeB")
            xoB = xbuf.tile([64, XW], bf16, tag="xoB")
            for t, d in ((xeA, xe), (xoA, xo), (xeB, xe2), (xoB, xo2)):
                nc.sync.dma_start(t[:], d[:])
            nc.sync.dma_start(wqt[:], wq[0:128, :])
            nc.sync.dma_start(wqt2[:], wq[128:192, :])
            lq_A = wt.tile([128, 256], bf16, tag="lqA")
            lq_B = wt.tile([65, 256], bf16, tag="lqB")
            lk_A = wt.tile([128, 256], bf16, tag="lkA")
            lk_B = wt.tile([65, 256], bf16, tag="lkB")
            rv_A = wt.tile([128, VW], bf16, tag="rvA")
            rv_B = wt.tile([65, VW], bf16, tag="rvB")
            for t, d in ((lq_A, lqA), (lq_B, lqB), (lk_A, lkA), (lk_B, lkB),
                         (rv_A, rvA), (rv_B, rvB)):
                nc.sync.dma_start(t[:], d[:])

            zqA = zp.tile([128, PADN], bf16, tag="zqA")
            zqB = zp.tile([65, PADN], bf16, tag="zqB")
            zkA = zp.tile([128, M], bf16, tag="zkA")
            zkB = zp.tile([65, M], bf16, tag="zkB")
            nc.vector.memset(zqB[64:65, :], 1.0)
            nc.vector.memset(zkB[64:65, :], 1.0)

            # ---- depthwise convs on DVE ----
            def dw_taps(zt, prow, weights, stride, xeT, xoT, out_ap, kv,
                        eng=None):
                eng = eng or nc.vector
                # taps: delta = (di*PAD + dj) relative to padded-coords cell
                first = True
                for di in range(3):
                    for dj in range(3):
                        k = 3 * di + dj
                        if kv:
                            d = di * PAD + dj
                        else:
                            d = (di - 1) * PAD + (dj - 1)
                        if d % 2 == 0:
                            src_t, off = xeT, XBASE + d
                        else:
                            src_t, off = xoT, XBASE + d - 1
                        if kv:
                            src = src_t[0:prow, off:off + 3248].rearrange(
                                "p (a b) -> p a b", a=MO)[:, :, 0:55:2]
                        else:
                            src = src_t[0:prow, off:off + PADN]
                        sc = weights[0:prow, k:k + 1]
                        if first:
                            eng.tensor_scalar(
                                out_ap, src, sc, None, AL.mult)
                            first = False
                        else:
                            eng.scalar_tensor_tensor(
                                out_ap, src, sc, out_ap, AL.mult, AL.add)

            def kv_taps(zt, prow, weights, xeT, xoT, kvt):
                dst = zt[0:prow, 0:M]
                pending = []
                for di in range(3):
                    for dj in range(3):
                        kk = 3 * di + dj
                        d = di * PAD + dj
                        if d % 2 == 0:
                            src_t, off = xeT, XBASE + d
                        else:
                            src_t, off = xoT, XBASE + d - 1
                        src = src_t[0:prow, off:off + 3248].rearrange(
                            "p (a b) -> p a b", a=MO)[:, :, 0:55:2]
                        sc = weights[0:prow, kk:kk + 1]
                        tp = kvt.tile([prow, M], bf16, tag="kvt")
                        nc.scalar.mul(
                            tp[:, :].rearrange("p (a b) -> p a b", a=MO),
                            src, sc)
                        pending.append(tp)
                        if len(pending) == 2 and kk == 1:
                            nc.vector.tensor_tensor(
                                dst, pending[0][:], pending[1][:],
                                AL.add)
                            pending = []
                        elif kk > 1:
                            nc.vector.tensor_tensor(
                                dst, dst, pending.pop()[:], AL.add)

            with tc.tile_pool(name="kvt", bufs=3) as kvtp:
                kv_taps(zkA, 128, wkt, xeA, xoA, kvtp)
                kv_taps(zkB, 64, wkt2, xeB, xoB, kvtp)

            # ---- k projection: kT [96, 784] x2 (head pairs) ----
            kTa = qk.tile([128, M], bf16, tag="kTa")
            kTb = qk.tile([128, M], bf16, tag="kTb")
            for mg, kt in ((0, kTa), (1, kTb)):
                for j in range(2):
                    pk = pso.tile([112, 392], mybir.dt.float32, tag="po")
                    sl = slice(392 * j, 392 * j + 392)
                    nc.tensor.matmul(pk[:], lk_A[:, 128 * mg:128 * mg + 112],
                                     zkA[:, sl], start=True, stop=False)
                    nc.tensor.matmul(pk[:], lk_B[:, 128 * mg:128 * mg + 112],
                                     zkB[:, sl], start=False, stop=True)
                    nc.scalar.copy(kt[0:112, sl], pk[:])

            # ---- v projection: interleaved v_aug [112, 196] x 7 ----
            vS = vsp.tile([MT, NMT * VW], bf16, tag="vS")
            for mt in range(NMT):
                pv = pso.tile([MT, VW], mybir.dt.float32, tag="po")
                nc.tensor.matmul(pv[:], zkA[:, MT * mt:MT * mt + MT],
                                 rv_A[:], start=True, stop=False)
                nc.tensor.matmul(pv[:], zkB[:, MT * mt:MT * mt + MT],
                                 rv_B[:], start=False, stop=True)
                nc.scalar.copy(vS[:, VW * mt:VW * mt + VW], pv[:])

            # ---- pipelined: per n-window dw_q -> q proj -> attention ----
            qTa = qk.tile([128, N], bf16, tag="qTa")
            qTb = qk.tile([128, N], bf16, tag="qTb")

            def dwq_window(c):
                # out cols [464c+58, 464c+522): image rows 8c..8c+8 (padded)
                lo = 464 * c + 58
                for prow, zt, xeT, xoT, wts, tmp in (
                        (128, zqA, xeA, xoA, wqt, tmpA),
                        (64, zqB, xeB, xoB, wqt2, tmpB)):
                    outw = zt[0:prow, lo:lo + 464]
                    first = True
                    for di in range(3):
                        for dj in range(3):
                            k = 3 * di + dj
                            d = (di - 1) * PAD + (dj - 1)
                            if d % 2 == 0:
                                src_t, off = xeT, XBASE + d + lo
                            else:
                                src_t, off = xoT, XBASE + d - 1 + lo
                            srcw = src_t[0:prow, off:off + 464]
                            sc = wts[0:prow, k:k + 1]
                            if first:
                                nc.vector.tensor_scalar(
                                    outw, srcw, sc, None, AL.mult)
                                first = False
                            else:
                                tw = tmp[0:prow, 0:464]
                                nc.vector.tensor_scalar(
                                    tw, srcw, sc, None, AL.mult)
                                nc.vector.tensor_tensor(
                                    outw, outw, tw, AL.add)

            def zq_chunk(zt, prow, c):
                base = 59 + 464 * c
                return zt[0:prow, base:base + 464].rearrange(
                    "p (a b) -> p a b", a=8)[:, :, 0:56]

            tmpA = tmpp.tile([128, 464], bf16, tag="tmpA")
            tmpB = tmpp.tile([64, 464], bf16, tag="tmpB")
            unda = undp.tile([128, N], mybir.dt.float32, tag="unda")
            undb = undp.tile([128, N], mybir.dt.float32, tag="undb")

            for w in range(NCH):
                dwq_window(w)
                for mg, qt in ((0, qTa), (1, qTb)):
                    pq = pso.tile([112, CHUNK], mybir.dt.float32, tag="po")
                    nc.tensor.matmul(pq[:], lq_A[:, 128 * mg:128 * mg + 112],
                                     zq_chunk(zqA, 128, w),
                                     start=True, stop=False)
                    nc.tensor.matmul(pq[:], lq_B[:, 128 * mg:128 * mg + 112],
                                     zq_chunk(zqB, 65, w),
                                     start=False, stop=True)
                    sl = slice(CHUNK * w, CHUNK * w + CHUNK)
                    nc.scalar.copy(qt[0:112, sl], pq[:])
                for hp, (kt, qt, und) in enumerate(
                        ((kTa, qTa, unda), (kTb, qTb, undb))):
                    po0 = pso.tile([49, CHUNK], mybir.dt.float32, tag="po")
                    po1 = pso.tile([49, CHUNK], mybir.dt.float32, tag="po")
                    for mt in range(NMT):
                        s01 = pss.tile([MT, 1024], mybir.dt.float32, tag="s")
                        nc.tensor.matmul(
                            s01[:, 0:CHUNK], kt[0:48, MT * mt:MT * mt + MT],
                            qt[0:48, CHUNK * w:CHUNK * w + CHUNK],
                            start=True, stop=True)
                        nc.tensor.matmul(
                            s01[:, 512:512 + CHUNK],
                            kt[64:112, MT * mt:MT * mt + MT],
                            qt[64:112, CHUNK * w:CHUNK * w + CHUNK],
                            start=True, stop=True)
                        p01 = ptp.tile([MT, 2 * CHUNK], bf16, tag="p0")
                        sview = s01[:, 0:1024].rearrange(
                            "p (a b) -> p a b", a=2)[:, :, 0:CHUNK]
                        pview = p01[:, 0:2 * CHUNK].rearrange(
                            "p (a b) -> p a b", a=2)
                        nc.scalar.activation(pview, sview, AF.Exp)
                        p0 = p01[:, 0:CHUNK]
                        p1 = p01[:, CHUNK:2 * CHUNK]
                        c0 = VW * mt + 49 * (2 * hp)
                        c1 = VW * mt + 49 * (2 * hp + 1)
                        nc.tensor.matmul(
                            po0[:], vS[:, c0:c0 + 49],
                            p0, start=(mt == 0), stop=(mt == NMT - 1))
                        nc.tensor.matmul(
                            po1[:], vS[:, c1:c1 + 49],
                            p1, start=(mt == 0), stop=(mt == NMT - 1))
                    nc.vector.tensor_copy(
                        und[0:49, CHUNK * w:CHUNK * w + CHUNK], po0[:])
                    nc.vector.tensor_copy(
                        und[64:113, CHUNK * w:CHUNK * w + CHUNK], po1[:])
                sl = slice(CHUNK * w, CHUNK * w + CHUNK)
                for und, ot in ((unda, oa), (undb, ob)):
                    dens = denp.tile([2, CHUNK], mybir.dt.float32, tag="dens")
                    for hh in range(2):
                        base = 64 * hh
                        nc.sync.dma_start(dens[hh:hh + 1, :],
                                          und[base + 48:base + 49, sl])
                    rc = fin2.tile([2, CHUNK], mybir.dt.float32, tag="rc")
                    nc.vector.reciprocal_approx_fast(rc[:], dens[:])
                    rcd = drp.tile([2, CHUNK], mybir.dt.float32, tag="rcd")
                    nc.sync.dma_start(rcd[:], rc[:])
                    for hh in range(2):
                        base = 64 * hh
                        rcb = rcbpool.tile([128, CHUNK], mybir.dt.float32,
                                           tag="rcb")
                        nc.sync.dma_start(
                            rcb[:], rcd[hh:hh + 1, :]
                            .partition_broadcast(128))
                        deng = nc.vector if w == NCH - 1 else nc.gpsimd
                        deng.tensor_tensor(
                            und[base:base + 48, sl],
                            und[base:base + 48, sl],
                            rcb[base:base + 48, :], AL.mult)
                        nc.sync.dma_start(ot[48 * hh:48 * hh + 48, sl],
                                          und[base:base + 48, sl])



    nc.finalize()
    return nc


def _pv_fix():
    pass


def _host_prep(x, H_, W_, dw_q, g_q, b_q, m_q, v_q, pw_q,
               dw_kv, g_kv, b_kv, m_kv, v_kv, pw_kv,
               Wq, bq, Wk, bk, Wv, bv):
    f64 = np.float64
    s_q = (g_q / np.sqrt(v_q + EPS)).astype(f64)
    t_q = b_q.astype(f64) - m_q.astype(f64) * s_q
    s_k = (g_kv / np.sqrt(v_kv + EPS)).astype(f64)
    t_k = b_kv.astype(f64) - m_kv.astype(f64) * s_k
    pq2 = pw_q[:, :, 0, 0].astype(f64)
    pkv2 = pw_kv[:, :, 0, 0].astype(f64)
    scale = HD ** -0.5

    Bq = (Wq.astype(f64) @ pq2) * s_q[None, :] * scale
    cq = (Wq.astype(f64) @ (pq2 @ t_q) + bq.astype(f64)) * scale
    Bk = (Wk.astype(f64) @ pkv2[:C]) * s_k[None, :]
    ck = Wk.astype(f64) @ (pkv2[:C] @ t_k) + bk.astype(f64)
    Bv = (Wv.astype(f64) @ pkv2[C:]) * s_k[None, :]
    cv = Wv.astype(f64) @ (pkv2[C:] @ t_k) + bv.astype(f64)

    def pad_lhsT(Bm, cvec):
        full = np.vstack([Bm.T, cvec[None, :]])  # [193, 192]
        padded = np.zeros((193, 256), np.float64)
        for mg in range(2):
            padded[:, 128 * mg + 0:128 * mg + 48] = full[:, 96 * mg + 0:96 * mg + 48]
            padded[:, 128 * mg + 64:128 * mg + 112] = full[:, 96 * mg + 48:96 * mg + 96]
        return padded[0:128].astype(BF16), padded[128:193].astype(BF16)

    lqA, lqB = pad_lhsT(Bq, cq)
    lkA, lkB = pad_lhsT(Bk, ck)

    rv = np.zeros((C, VW), f64)
    rb = np.zeros((1, VW), f64)
    for h in range(NH):
        rv[:, 49 * h:49 * h + 48] = Bv.T[:, 48 * h:48 * h + 48]
        rb[0, 49 * h:49 * h + 48] = cv[48 * h:48 * h + 48]
        rb[0, 49 * h + 48] = 1.0
    rvA = rv[0:128].astype(BF16)
    rvB = np.vstack([rv[128:192], rb]).astype(BF16)

    wqc = dw_q[:, 0].reshape(C, 9).astype(np.float32)
    wkc = dw_kv[:, 0].reshape(C, 9).astype(np.float32)

    B = x.shape[0]
    xpads = []
    for b in range(B):
        xb = np.ascontiguousarray(x[b].T).reshape(C, H, W)
        xp = np.zeros((C, PAD, PAD), np.float32)
        xp[:, 1:-1, 1:-1] = xb
        flat = np.zeros((C, XW), np.float32)
        flat[:, XBASE:XBASE + PADN] = xp.reshape(C, PADN)
        xe = flat.astype(BF16)
        xo = np.zeros_like(xe)
        xo[:, 0:XW - 1] = xe[:, 1:XW]
        xpads.append((xe, xo))
    return (lqA, lqB, lkA, lkB, rvA, rvB, wqc, wkc, xpads)


def _run(inputs, trace=False, tmpdir=None):
    x = np.asarray(inputs["x"], np.float32)
    B = x.shape[0]
    prep = _host_prep(
        x, inputs["H"], inputs["W"], *[np.asarray(inputs[k], np.float32) for k in (
            "dw_q", "bn_q_gamma", "bn_q_beta", "bn_q_mean", "bn_q_var", "pw_q",
            "dw_kv", "bn_kv_gamma", "bn_kv_beta", "bn_kv_mean", "bn_kv_var",
            "pw_kv", "Wq", "bq", "Wk", "bk", "Wv", "bv")])
    lqA, lqB, lkA, lkB, rvA, rvB, wqc, wkc, xpads = prep

    if "nc" not in _CACHE:
        _CACHE["nc"] = _build_bass()
    nc = _CACHE["nc"]

    in_maps = []
    for b in range(B):
        xe, xo = xpads[b]
        in_maps.append({
            "xe": np.ascontiguousarray(xe[0:128]),
            "xo": np.ascontiguousarray(xo[0:128]),
            "xe2": np.ascontiguousarray(xe[128:192]),
            "xo2": np.ascontiguousarray(xo[128:192]),
            "lqA": lqA, "lqB": lqB, "lkA": lkA, "lkB": lkB,
            "rvA": rvA, "rvB": rvB, "wq": wqc, "wkv": wkc,
        })
    bkr = run_bass_kernel_spmd(nc, in_maps, list(range(B)),
                               trace=trace, tmpdir=tmpdir)
    res = bkr.results
    out = np.empty((B, N, C), np.float32)
    for b in range(B):
        oT = np.concatenate([res[b]["oa"], res[b]["ob"]], axis=0)
        out[b] = oT.T
    return out, bkr


def kernel(**inputs):
    return _run(inputs)[0]

